# revision 2
# baseline (speedup 1.0000x reference)
"""Trainium2 Bass kernel: per-sample hypernetwork depthwise 3x3 conv (bf16).

Reference computation (per batch b):
    W_dw[b] = (z[b] @ W_lin.T).reshape(OUT_C, 1, 3, 3)
    y[b]    = depthwise_conv2d(x[b], W_dw[b], padding=1)

Sharding: data-parallel over batch across 8 NeuronCores (2 batches/core),
W_lin replicated. Each core computes its own W_dw on-device.

Per-core design (bf16 data path; tolerance 2e-2 >> bf16 error ~3e-3):
  - x is downcast to bf16 on host; y is produced bf16 and upcast on host.
    Halves HBM traffic vs f32: ~33.5 MB/core -> ~93 us DMA floor.
  - channels (256) -> 2 groups of 128 on SBUF partitions
  - image split into 64-row bands; each band DMA'd as one contiguous
    transfer into an UNPADDED flat tile (rows back-to-back, pad offset
    P_OFF). Width-edge taps wrap into the neighboring row; those wrong
    contributions are subtracted by small stride-W "correction" FMAs with
    negated weights reading the exact same wrapped positions.
  - 9 conv taps split between engines per band (alternating 4-PE/5-DVE,
    3-PE/6-DVE):
      * PE taps: diagonal-weight bf16 matmuls accumulating in PSUM f32
        (1 cycle/row at free size 512)
      * ACT drains each 16-row PSUM group into the bf16 band output tile
      * DVE taps: band-level (8192-elem) bf16 scalar_tensor_tensor FMAs
        in-place on the output tile -- all-bf16 SBUF operands hit the
        DVE 4x_2p mode (4 elem/cycle)
  - input DMAs on the SP (sync) HWDGE ring, output DMAs on the ACT ring
  - W_dw computed on-device by 18 small fp32 matmuls from a host-side
    re-layout of W_lin (pure permutation/transpose, no host math)
"""

import os
import sys

for _p in ("/opt/trn_rl_repo", "/root/.axon_site", "/root/.axon_site/_ro/trn_rl_repo",
           "/root/.axon_site/_ro/pypackages"):
    if os.path.isdir(_p) and _p not in sys.path:
        sys.path.append(_p)

import numpy as np
import ml_dtypes

import concourse.bass as bass
import concourse.tile as tile
from concourse import bacc, mybir
from concourse import bass_utils
from concourse.alu_op_type import AluOpType

F32 = mybir.dt.float32
BF16 = mybir.dt.bfloat16

# problem constants (hardcoded per contract)
B, OUT_C, H, W = 16, 256, 128, 128
K, Z_DIM = 3, 64
N_CORES = 8
B_PER = B // N_CORES          # 2 batches per core
G = OUT_C // 128              # 2 channel groups of 128

# tuning knobs
PE_TAP_PATTERN = (4, 3)       # PE taps per band, alternating
ROWS_BAND = 64
ROWS_GROUP = 16
XPOOL_BUFS = 4
OPOOL_BUFS = 3
P_OFF = 2                     # leading pad elems in flat tile (4B aligned)

TAPS = [(dy, dx) for dy in range(3) for dx in range(3)]


def build_nc(pe_pattern=PE_TAP_PATTERN, rows_band=ROWS_BAND, rows_group=ROWS_GROUP,
             b_per=B_PER, h=H):
    """Build the per-core Bass program. Returns compiled Bacc object."""
    n_bands = h // rows_band
    grp_per_band = rows_band // rows_group
    banks_per_grp = rows_group * W // 512
    rows_bank = 512 // W
    grp_free = rows_group * W
    band_free = rows_band * W
    pad_rows = rows_band + 2
    # data rows t at [P_OFF + t*W, P_OFF + (t+1)*W); wrap elems at P_OFF-1
    # and P_OFF + pad_rows*W; extra W slack so stride-W correction APs stay
    # in range
    flat_n = P_OFF + pad_rows * W + 1 + W + 1
    max_pe = max(pe_pattern)

    nc = bacc.Bacc("TRN2", target_bir_lowering=False, debug=False)

    x_d = nc.dram_tensor("x", [b_per, OUT_C, h, W], BF16, kind="ExternalInput")
    zt_d = nc.dram_tensor("zT", [Z_DIM, b_per], F32, kind="ExternalInput")
    wlt_d = nc.dram_tensor("wlt", [Z_DIM, OUT_C * K * K], F32, kind="ExternalInput")
    ident_d = nc.dram_tensor("ident", [128, 128], F32, kind="ExternalInput")
    zeros_d = nc.dram_tensor("zeros", [128, W], BF16, kind="ExternalInput")
    y_d = nc.dram_tensor("y", [b_per, OUT_C, h, W], BF16, kind="ExternalOutput")

    n_chunks = OUT_C * K * K // 128          # 18
    wd_cols = K * K * G * b_per              # 36, col = (g*9 + t)*b_per + b

    with tile.TileContext(nc) as tc:
        with tc.tile_pool(name="wconst", bufs=1) as wpool:
            ident = wpool.tile([128, 128], F32)
            nc.sync.dma_start(ident[:], ident_d.ap()[:, :])
            wlt = wpool.tile([Z_DIM, OUT_C * K * K], F32)
            half = OUT_C * K * K // 2
            nc.sync.dma_start(wlt[:, 0:half], wlt_d.ap()[:, 0:half])
            nc.sync.dma_start(wlt[:, half:], wlt_d.ap()[:, half:])
            zt = wpool.tile([Z_DIM, b_per], F32)
            nc.sync.dma_start(zt[:], zt_d.ap()[:, :])

            wd = wpool.tile([128, wd_cols], F32)
            with tc.tile_pool(name="wpsum", bufs=2, space="PSUM") as wps:
                for j in range(n_chunks):
                    ps = wps.tile([128, b_per], F32)
                    nc.tensor.matmul(ps[:], wlt[:, 128 * j:128 * (j + 1)], zt[:],
                                     start=True, stop=True)
                    nc.scalar.copy(wd[:, b_per * j:b_per * (j + 1)], ps[:])

            # bf16 diagonal weight matrices for all taps that may run on PE
            diags = {}
            for b in range(b_per):
                for g in range(G):
                    for ti in range(max_pe):
                        col = (g * K * K + ti) * b_per + b
                        dtile = wpool.tile([128, 128], BF16, tag=f"diag_{b}_{g}_{ti}")
                        nc.scalar.mul(dtile[:], ident[:], wd[:, col:col + 1])
                        diags[(b, g, ti)] = dtile

            # bf16 per-partition weights for DVE taps, and negated ones for
            # the wrap corrections
            wdh = wpool.tile([128, wd_cols], BF16)
            nc.scalar.copy(wdh[:], wd[:])
            wdn = wpool.tile([128, wd_cols], BF16)
            nc.scalar.mul(wdn[:], wd[:], -1.0)

            with tc.tile_pool(name="xband", bufs=XPOOL_BUFS) as xpool, \
                 tc.tile_pool(name="oband", bufs=OPOOL_BUFS) as opool, \
                 tc.tile_pool(name="psum", bufs=2, space="PSUM") as pspool:
                band_no = 0
                for b in range(b_per):
                    for g in range(G):
                        for band in range(n_bands):
                            r0 = band * rows_band
                            lo = max(r0 - 1, 0)
                            hi = min(r0 + rows_band + 1, h)
                            # tile row t holds image row r0-1+t at flat
                            # [P_OFF + t*W, P_OFF + (t+1)*W)
                            xt = xpool.tile([128, flat_n], BF16)
                            # zero wrap-pad elems once per pool buffer; later
                            # reuses hold stale-but-finite values that cancel
                            # exactly against the corrections
                            if band_no < XPOOL_BUFS:
                                for off in (P_OFF - 1, P_OFF + pad_rows * W):
                                    nc.vector.memset(xt[:, off:off + 1], 0.0)
                            if r0 == 0:
                                nc.scalar.dma_start(xt[:, P_OFF:P_OFF + W],
                                                    zeros_d.ap()[:, :])
                            if r0 + rows_band == h:
                                nc.scalar.dma_start(
                                    xt[:, P_OFF + (pad_rows - 1) * W:
                                       P_OFF + pad_rows * W],
                                    zeros_d.ap()[:, :])
                            dst0 = P_OFF + (lo - (r0 - 1)) * W
                            nc.sync.dma_start(
                                xt[:, dst0:dst0 + (hi - lo) * W],
                                x_d.ap()[b, 128 * g:128 * (g + 1), lo:hi, :])

                            ot = opool.tile([128, band_free], BF16)
                            n_pe = pe_pattern[band_no % len(pe_pattern)]
                            band_no += 1
                            pe_taps = TAPS[:n_pe]
                            dve_taps = TAPS[n_pe:]

                            for grp in range(grp_per_band):
                                gr0 = grp * rows_group
                                ps = pspool.tile([128, grp_free], F32)
                                for ti in range(n_pe):
                                    dy, dx = pe_taps[ti]
                                    for bank in range(banks_per_grp):
                                        s = (P_OFF - 1
                                             + (gr0 + bank * rows_bank + dy) * W
                                             + dx)
                                        nc.tensor.matmul(
                                            ps[:, 512 * bank:512 * (bank + 1)],
                                            diags[(b, g, ti)][:],
                                            xt[:, s:s + 512],
                                            start=(ti == 0),
                                            stop=(ti == n_pe - 1))
                                # ACT drains this group's PSUM into the bf16
                                # band output tile
                                nc.scalar.copy(ot[:, gr0 * W:gr0 * W + grp_free],
                                               ps[:])

                            # DVE taps over the whole band, in place on ot:
                            # all-bf16 SBUF operands -> 4x_2p mode
                            for k, (dy, dx) in enumerate(dve_taps):
                                ti = n_pe + k
                                col = (g * K * K + ti) * b_per + b
                                s = P_OFF - 1 + dy * W + dx
                                nc.vector.scalar_tensor_tensor(
                                    out=ot[:], in0=xt[:, s:s + band_free],
                                    scalar=wdh[:, col:col + 1], in1=ot[:],
                                    op0=AluOpType.mult, op1=AluOpType.add)

                            # width-edge wrap corrections over the whole band:
                            # ot[r, 0]   -= w[dy,0] * flat[P-1 + (r+dy)*W]
                            # ot[r, W-1] -= w[dy,2] * flat[P + (r+dy+1)*W]
                            otv = ot[:].rearrange("p (r c) -> p r c", c=W)
                            for dy in range(3):
                                for dx, (off, oc) in (
                                        (0, (P_OFF - 1 + dy * W, 0)),
                                        (2, (P_OFF + (dy + 1) * W, W - 1))):
                                    ti = dy * 3 + dx
                                    col = (g * K * K + ti) * b_per + b
                                    in0 = (xt[:, off:off + rows_band * W]
                                           .rearrange("p (r c) -> p r c", c=W)
                                           [:, :, 0:1])
                                    oe = otv[:, :, oc:oc + 1]
                                    nc.vector.scalar_tensor_tensor(
                                        out=oe, in0=in0,
                                        scalar=wdn[:, col:col + 1], in1=oe,
                                        op0=AluOpType.mult, op1=AluOpType.add)

                            # output DMA on the ACT HWDGE ring
                            nc.scalar.dma_start(
                                y_d.ap()[b, 128 * g:128 * (g + 1),
                                         r0:r0 + rows_band, :],
                                ot[:])

    nc.compile()
    return nc


def make_in_maps(x, z, W_lin, b_per=B_PER):
    """Host-side shard + layout/dtype transforms (no math)."""
    wl = np.asarray(W_lin, dtype=np.float32)
    wlperm = (wl.reshape(G, 128, K * K, Z_DIM)
                .transpose(0, 2, 1, 3)
                .reshape(OUT_C * K * K, Z_DIM))
    wlt = np.ascontiguousarray(wlperm.T)                  # [64, 2304]
    ident = np.eye(128, dtype=np.float32)
    x = np.asarray(x, dtype=np.float32)
    z = np.asarray(z, dtype=np.float32)
    xh = x.astype(ml_dtypes.bfloat16)
    in_maps = []
    for c in range(N_CORES):
        sl = slice(c * b_per, (c + 1) * b_per)
        in_maps.append({
            "x": np.ascontiguousarray(xh[sl]),
            "zT": np.ascontiguousarray(z[sl].T),          # [64, b_per]
            "wlt": wlt,
            "ident": ident,
            "zeros": np.zeros((128, W), dtype=ml_dtypes.bfloat16),
        })
    return in_maps


_NC_CACHE = {}


def kernel(x, z, W_lin):
    key = "main"
    if key not in _NC_CACHE:
        _NC_CACHE[key] = build_nc()
    nc = _NC_CACHE[key]
    in_maps = make_in_maps(x, z, W_lin)
    res = bass_utils.run_bass_kernel_spmd(nc, in_maps, core_ids=list(range(N_CORES)))
    out = np.concatenate([res.results[c]["y"] for c in range(N_CORES)], axis=0)
    return out.astype(np.float32)


# revision 6
# speedup vs baseline: 1.7758x; 1.7758x over previous
"""Trainium2 Bass kernel: per-sample hypernetwork depthwise 3x3 conv (bf16).

Reference computation (per batch b):
    W_dw[b] = (z[b] @ W_lin.T).reshape(OUT_C, 1, 3, 3)
    y[b]    = depthwise_conv2d(x[b], W_dw[b], padding=1)

Sharding: data-parallel over batch across 8 NeuronCores (2 batches/core),
W_lin replicated. Each core computes its own W_dw on-device.

Per-core design (bf16 data path; tolerance 2e-2 >> bf16 error ~1e-2):
  - x is downcast to bf16 on host; y is produced bf16 and upcast on host.
    Halves HBM traffic vs f32: ~33.5 MB/core -> ~93 us DMA floor.
  - channels (256) -> 2 groups of 128 on SBUF partitions
  - image split into 64-row bands; each band DMA'd as one contiguous
    transfer into an UNPADDED flat tile (rows back-to-back, pad offset
    P_OFF). Width-edge taps wrap into the neighboring row; those wrong
    contributions are subtracted by small stride-W "correction" FMAs with
    negated weights reading the exact same wrapped positions.
  - 9 conv taps split across ALL FOUR compute engines per band
    (fused DVE scalar_tensor_tensor runs at 1x only -- no 2x/4x perf
    modes for the STT encoding -- so DVE taps are split into
    tensor_scalar mul at 4x + tensor_tensor add at 2x):
      * PE: N_PE taps as diagonal-weight bf16 matmuls -> PSUM f32
        (1 cycle/row, 0.42 ns/elem/tap)
      * ACT: drains each 16-row PSUM group into the bf16 band output ot,
        plus N_ACT taps tmp = w*x via scalar.mul (0.83 ns/elem/pass)
      * DVE: N_DVE taps as tt = w*x (tensor_scalar, 4x = 0.26) then
        ot += tt (tensor_tensor add, 2x = 0.52); also merges ACT tmps
      * Pool/GpSimd: N_POOL taps as in-place STT on ot (~1.4 ns/elem)
        plus the 6 small wrap-correction FMAs per band
  - input DMAs on the SP (sync) HWDGE ring, output DMAs on the ACT ring
  - W_dw computed on-device by 18 small fp32 matmuls from a host-side
    re-layout of W_lin (pure permutation/transpose, no host math)
"""

import os
import sys

for _p in ("/opt/trn_rl_repo", "/root/.axon_site", "/root/.axon_site/_ro/trn_rl_repo",
           "/root/.axon_site/_ro/pypackages"):
    if os.path.isdir(_p) and _p not in sys.path:
        sys.path.append(_p)

import numpy as np
import ml_dtypes

import concourse.bass as bass
import concourse.tile as tile
from concourse import bacc, mybir
from concourse import bass_utils
from concourse.alu_op_type import AluOpType

F32 = mybir.dt.float32
BF16 = mybir.dt.bfloat16

# problem constants (hardcoded per contract)
B, OUT_C, H, W = 16, 256, 128, 128
K, Z_DIM = 3, 64
N_CORES = 8
B_PER = B // N_CORES          # 2 batches per core
G = OUT_C // 128              # 2 channel groups of 128

# tuning knobs: per-band tap counts (pe, act, dve), cycled per band
TAP_PLAN = ((5, 2, 2), (6, 2, 1), (5, 1, 3), (6, 2, 1))
ROWS_BAND = 64
ROWS_GROUP = 16
XPOOL_BUFS = 3
OPOOL_BUFS = 3
APOOL_BUFS = 2
VPOOL_BUFS = 3
P_OFF = 2                     # leading pad elems in flat tile (4B aligned)
CORR_ENGINE = "vector"        # engine for wrap corrections

TAPS = [(dy, dx) for dy in range(3) for dx in range(3)]


def build_nc(tap_plan=TAP_PLAN, rows_band=ROWS_BAND, rows_group=ROWS_GROUP,
             b_per=B_PER, h=H):
    """Build the per-core Bass program. Returns compiled Bacc object."""
    n_bands = h // rows_band
    grp_per_band = rows_band // rows_group
    banks_per_grp = rows_group * W // 512
    rows_bank = 512 // W
    grp_free = rows_group * W
    band_free = rows_band * W
    pad_rows = rows_band + 2
    # data rows t at [P_OFF + t*W, P_OFF + (t+1)*W); wrap elems at P_OFF-1
    # and P_OFF + pad_rows*W; extra W slack so stride-W correction APs stay
    # in range
    flat_n = P_OFF + pad_rows * W + 1 + W + 1
    max_pe = max(p[0] for p in tap_plan)

    nc = bacc.Bacc("TRN2", target_bir_lowering=False, debug=False)

    x_d = nc.dram_tensor("x", [b_per, OUT_C, h, W], BF16, kind="ExternalInput")
    zt_d = nc.dram_tensor("zT", [Z_DIM, b_per], F32, kind="ExternalInput")
    wlt_d = nc.dram_tensor("wlt", [Z_DIM, OUT_C * K * K], F32, kind="ExternalInput")
    ident_d = nc.dram_tensor("ident", [128, 128], F32, kind="ExternalInput")
    zeros_d = nc.dram_tensor("zeros", [128, W], BF16, kind="ExternalInput")
    y_d = nc.dram_tensor("y", [b_per, OUT_C, h, W], BF16, kind="ExternalOutput")

    n_chunks = OUT_C * K * K // 128          # 18
    wd_cols = K * K * G * b_per              # 36, col = (g*9 + t)*b_per + b

    with tile.TileContext(nc) as tc:
        with tc.tile_pool(name="wconst", bufs=1) as wpool:
            ident = wpool.tile([128, 128], F32)
            nc.sync.dma_start(ident[:], ident_d.ap()[:, :])
            wlt = wpool.tile([Z_DIM, OUT_C * K * K], F32)
            half = OUT_C * K * K // 2
            nc.sync.dma_start(wlt[:, 0:half], wlt_d.ap()[:, 0:half])
            nc.sync.dma_start(wlt[:, half:], wlt_d.ap()[:, half:])
            zt = wpool.tile([Z_DIM, b_per], F32)
            nc.sync.dma_start(zt[:], zt_d.ap()[:, :])

            wd = wpool.tile([128, wd_cols], F32)
            with tc.tile_pool(name="wpsum", bufs=2, space="PSUM") as wps:
                for j in range(n_chunks):
                    ps = wps.tile([128, b_per], F32)
                    nc.tensor.matmul(ps[:], wlt[:, 128 * j:128 * (j + 1)], zt[:],
                                     start=True, stop=True)
                    nc.scalar.copy(wd[:, b_per * j:b_per * (j + 1)], ps[:])

            # bf16 diagonal weight matrices for all taps that may run on PE
            diags = {}
            for b in range(b_per):
                for g in range(G):
                    for ti in range(max_pe):
                        col = (g * K * K + ti) * b_per + b
                        dtile = wpool.tile([128, 128], BF16, tag=f"diag_{b}_{g}_{ti}")
                        nc.scalar.mul(dtile[:], ident[:], wd[:, col:col + 1])
                        diags[(b, g, ti)] = dtile

            # bf16 per-partition weights, and negated ones for the wrap
            # corrections
            wdn = wpool.tile([128, wd_cols], F32)
            nc.scalar.mul(wdn[:], wd[:], -1.0)

            corr_eng = getattr(nc, CORR_ENGINE)

            with tc.tile_pool(name="xband", bufs=XPOOL_BUFS) as xpool, \
                 tc.tile_pool(name="oband", bufs=OPOOL_BUFS) as opool, \
                 tc.tile_pool(name="atmp", bufs=APOOL_BUFS) as apool, \
                 tc.tile_pool(name="vtmp", bufs=VPOOL_BUFS) as vpool, \
                 tc.tile_pool(name="psum", bufs=2, space="PSUM") as pspool:
                band_no = 0
                for b in range(b_per):
                    for g in range(G):
                        for band in range(n_bands):
                            r0 = band * rows_band
                            lo = max(r0 - 1, 0)
                            hi = min(r0 + rows_band + 1, h)
                            # tile row t holds image row r0-1+t at flat
                            # [P_OFF + t*W, P_OFF + (t+1)*W)
                            xt = xpool.tile([128, flat_n], BF16)
                            # zero wrap-pad elems once per pool buffer; later
                            # reuses hold stale-but-finite values that cancel
                            # exactly against the corrections
                            if band_no < XPOOL_BUFS:
                                for off in (P_OFF - 1, P_OFF + pad_rows * W):
                                    nc.vector.memset(xt[:, off:off + 1], 0.0)
                            if r0 == 0:
                                nc.scalar.dma_start(xt[:, P_OFF:P_OFF + W],
                                                    zeros_d.ap()[:, :])
                            if r0 + rows_band == h:
                                nc.scalar.dma_start(
                                    xt[:, P_OFF + (pad_rows - 1) * W:
                                       P_OFF + pad_rows * W],
                                    zeros_d.ap()[:, :])
                            dst0 = P_OFF + (lo - (r0 - 1)) * W
                            nc.sync.dma_start(
                                xt[:, dst0:dst0 + (hi - lo) * W],
                                x_d.ap()[b, 128 * g:128 * (g + 1), lo:hi, :])

                            ot = opool.tile([128, band_free], BF16)
                            n_pe, n_act, n_dve = \
                                tap_plan[band_no % len(tap_plan)]
                            assert n_pe + n_act + n_dve == K * K
                            band_no += 1

                            def tap_col(ti, g=g, b=b):
                                return (g * K * K + ti) * b_per + b

                            def tap_src(ti):
                                dy, dx = TAPS[ti]
                                return P_OFF - 1 + dy * W + dx

                            # --- PE taps: accumulate in PSUM per 16-row group,
                            # drained to ot (bf16) by ACT
                            for grp in range(grp_per_band):
                                gr0 = grp * rows_group
                                ps = pspool.tile([128, grp_free], F32)
                                for ti in range(n_pe):
                                    dy, dx = TAPS[ti]
                                    for bank in range(banks_per_grp):
                                        s = (P_OFF - 1
                                             + (gr0 + bank * rows_bank + dy) * W
                                             + dx)
                                        nc.tensor.matmul(
                                            ps[:, 512 * bank:512 * (bank + 1)],
                                            diags[(b, g, ti)][:],
                                            xt[:, s:s + 512],
                                            start=(ti == 0),
                                            stop=(ti == n_pe - 1))
                                nc.scalar.copy(ot[:, gr0 * W:gr0 * W + grp_free],
                                               ps[:])

                            # --- ACT + DVE product taps (independent of ot)
                            merge_tiles = []
                            for i in range(n_act):
                                ti = n_pe + i
                                tmp = apool.tile([128, band_free], BF16)
                                nc.scalar.mul(tmp[:], xt[:, tap_src(ti):
                                                         tap_src(ti) + band_free],
                                              wd[:, tap_col(ti):tap_col(ti) + 1])
                                merge_tiles.append(tmp)
                            for i in range(n_dve):
                                ti = n_pe + n_act + i
                                tt = vpool.tile([128, band_free], BF16)
                                nc.vector.tensor_scalar(
                                    out=tt[:],
                                    in0=xt[:, tap_src(ti):tap_src(ti) + band_free],
                                    scalar1=wd[:, tap_col(ti):tap_col(ti) + 1],
                                    scalar2=None, op0=AluOpType.mult)
                                merge_tiles.append(tt)

                            # --- DVE merges product tiles into ot (2x TT adds)
                            for t in merge_tiles:
                                nc.vector.tensor_tensor(
                                    out=ot[:], in0=ot[:], in1=t[:],
                                    op=AluOpType.add)

                            # width-edge wrap corrections over the whole band:
                            # ot[r, 0]   -= w[dy,0] * flat[P-1 + (r+dy)*W]
                            # ot[r, W-1] -= w[dy,2] * flat[P + (r+dy+1)*W]
                            otv = ot[:].rearrange("p (r c) -> p r c", c=W)
                            for dy in range(3):
                                for dx, (off, oc) in (
                                        (0, (P_OFF - 1 + dy * W, 0)),
                                        (2, (P_OFF + (dy + 1) * W, W - 1))):
                                    ti = dy * 3 + dx
                                    in0 = (xt[:, off:off + rows_band * W]
                                           .rearrange("p (r c) -> p r c", c=W)
                                           [:, :, 0:1])
                                    oe = otv[:, :, oc:oc + 1]
                                    corr_eng.scalar_tensor_tensor(
                                        out=oe, in0=in0,
                                        scalar=wdn[:, tap_col(ti):tap_col(ti) + 1],
                                        in1=oe,
                                        op0=AluOpType.mult, op1=AluOpType.add)

                            # output DMA on the ACT HWDGE ring
                            nc.scalar.dma_start(
                                y_d.ap()[b, 128 * g:128 * (g + 1),
                                         r0:r0 + rows_band, :],
                                ot[:])

    nc.compile()
    return nc


def make_in_maps(x, z, W_lin, b_per=B_PER):
    """Host-side shard + layout/dtype transforms (no math)."""
    wl = np.asarray(W_lin, dtype=np.float32)
    wlperm = (wl.reshape(G, 128, K * K, Z_DIM)
                .transpose(0, 2, 1, 3)
                .reshape(OUT_C * K * K, Z_DIM))
    wlt = np.ascontiguousarray(wlperm.T)                  # [64, 2304]
    ident = np.eye(128, dtype=np.float32)
    x = np.asarray(x, dtype=np.float32)
    z = np.asarray(z, dtype=np.float32)
    xh = x.astype(ml_dtypes.bfloat16)
    in_maps = []
    for c in range(N_CORES):
        sl = slice(c * b_per, (c + 1) * b_per)
        in_maps.append({
            "x": np.ascontiguousarray(xh[sl]),
            "zT": np.ascontiguousarray(z[sl].T),          # [64, b_per]
            "wlt": wlt,
            "ident": ident,
            "zeros": np.zeros((128, W), dtype=ml_dtypes.bfloat16),
        })
    return in_maps


_NC_CACHE = {}


def kernel(x, z, W_lin):
    key = "main"
    if key not in _NC_CACHE:
        _NC_CACHE[key] = build_nc()
    nc = _NC_CACHE[key]
    in_maps = make_in_maps(x, z, W_lin)
    res = bass_utils.run_bass_kernel_spmd(nc, in_maps, core_ids=list(range(N_CORES)))
    out = np.concatenate([res.results[c]["y"] for c in range(N_CORES)], axis=0)
    return out.astype(np.float32)
